# revision 1
# baseline (speedup 1.0000x reference)
"""Trainium2 Bass kernel for nn_BiLSTMWithLM (B=64, T=1024, D_IN=400).

Data-parallel over batch: 8 cores x 8 sequences each.
  P1: bulk layer-0 input projections (bf16 matmuls, biases folded via an
      augmented ones-row), stored [p, gate, t*8+b] in DRAM.
  S0: layer-0 bidirectional LSTM scan. Per step: identity-matmul preloads
      PSUM with the input projection, 8 recurrent matmuls (4 gates x 2 dirs,
      bf16 stationary w_hhT) accumulate, ACT tanh/sigmoid, DVE cell update.
      Gate order permuted to [i,f,o,g] on host so sigmoid covers one slab.
  P2: layer-1 input projections from layer-0 output.
  S1: layer-1 scan.
  P3: head. BN1/linear/BN2 folded on host into LW/LB; computes
      u = tanh(LW @ l1out + LB) and the logit-difference drive
      du = w3s . u + K0 (written as [b, t]).
  P4: context scan reformulated as a scalar recurrence on the logit diff
      d_t = du_t + g*d_{t-1} - dl*sp(d_{t-1}) + a*d_{t-2} - b*sp(d_{t-2}),
      solved by Jacobi fixed-point iteration (contraction ~0.085/iter);
      lo0 = -softplus(d), lo1 = d - softplus(d).
"""
import os
import sys

sys.path.insert(0, "/opt/trn_rl_repo")

import numpy as np
import ml_dtypes

import concourse.bass as bass
import concourse.bacc as bacc
import concourse.mybir as mybir
from concourse import tile
from concourse.bass_utils import run_bass_kernel_spmd
from concourse.kernels.tile_matmul import matmul_tile_kernel
from contextlib import ExitStack

BF16 = mybir.dt.bfloat16
F32 = mybir.dt.float32
AF = mybir.ActivationFunctionType
OP = mybir.AluOpType

B, D_IN, H = 64, 400, 128
T = int(os.environ.get("KERNEL_T", "1024"))
N_CORES = 8
BL = B // N_CORES          # 8 local sequences
N = T * BL                 # columns, n = t*8 + b
C = 64                     # scan chunk (steps per DMA chunk)
N_JACOBI = 8
EPS = 1e-5


def _bf16(x):
    return np.asarray(x, dtype=ml_dtypes.bfloat16)


def _perm_gates(w):
    i, f, g, o = np.split(np.asarray(w), 4, axis=0)
    return np.concatenate([i, f, o, g], axis=0)


_BUILD_CACHE = {}


def _build():
    if T in _BUILD_CACHE:
        return _BUILD_CACHE[T]

    nc = bacc.Bacc("TRN2", target_bir_lowering=False, debug=False,
                   num_devices=N_CORES)

    def din(name, shape, dtype):
        return nc.dram_tensor(name, shape, dtype, kind="ExternalInput").ap()

    def dscratch(name, shape, dtype):
        return nc.dram_tensor(name, shape, dtype).ap()

    # inputs
    xk = din("xk", [128, 4, N], BF16)               # aug x, kxn for P1
    w0 = {d: din(f"w0{d}", [128, 4, 512], BF16) for d in "fb"}
    w1 = {d: din(f"w1{d}", [128, 3, 512], BF16) for d in "fb"}
    whh0 = {d: din(f"whh0{d}", [128, 512], BF16) for d in "fb"}
    whh1 = {d: din(f"whh1{d}", [128, 512], BF16) for d in "fb"}
    lwk = din("lwk", [128, 2, 64], BF16)            # LW.T tiled
    lbv = din("lbv", [64, 1], F32)                  # LB bias
    w3s = din("w3s", [64, 1], BF16)                 # head diff vector
    coef = din("coef", [8, 8], F32)                 # [g, -dl, a, -b, K0]
    ident = din("ident", [128, 128], BF16)
    outv = nc.dram_tensor("outv", [N, 2], F32, kind="ExternalOutput").ap()

    # scratch
    proj0 = {d: dscratch(f"proj0{d}", [128, 4, N], BF16) for d in "fb"}
    proj1 = {d: dscratch(f"proj1{d}", [128, 4, N], BF16) for d in "fb"}
    l0out = dscratch("l0out", [128, 3, N], BF16)
    l1out = dscratch("l1out", [128, 2, N], BF16)
    dud = dscratch("dud", [BL, T], F32)             # du as [b, t]

    with tile.TileContext(nc) as tc:
        # ---- init: l0out kb=2 block (ones row at p=0, zeros elsewhere) ----
        with ExitStack() as ctx:
            pool = ctx.enter_context(tc.tile_pool(name="initp", bufs=1))
            ozt = pool.tile([128, 512], BF16)
            nc.vector.memset(ozt[:], 0.0)
            nc.vector.memset(ozt[0:1, :], 1.0)
            for i in range(N // 512):
                nc.sync.dma_start(l0out[:, 2, bass.ts(i, 512)], ozt[:])

        # ---- P1: layer-0 projections ----
        with ExitStack() as ctx:
            for d in "fb":
                matmul_tile_kernel(tc, w0[d][:], xk[:], proj0[d][:])

        # ---- scan helper ----
        def scan(layer, projf, projb, whhf_d, whhb_d, out_ap, kb_f, kb_b):
            with ExitStack() as ctx:
                cpool = ctx.enter_context(tc.tile_pool(name=f"wh{layer}", bufs=1))
                whf = cpool.tile([128, 512], BF16)
                whb = cpool.tile([128, 512], BF16)
                idt = cpool.tile([128, 128], BF16)
                nc.sync.dma_start(whf[:], whhf_d[:])
                nc.sync.dma_start(whb[:], whhb_d[:])
                nc.sync.dma_start(idt[:], ident[:])

                ppool = ctx.enter_context(tc.tile_pool(name=f"pj{layer}", bufs=2))
                hpool = ctx.enter_context(tc.tile_pool(name=f"hc{layer}", bufs=2))
                spool = ctx.enter_context(tc.tile_pool(name=f"s{layer}", bufs=3))
                cstp = ctx.enter_context(tc.tile_pool(name=f"cst{layer}", bufs=2))
                psum = ctx.enter_context(
                    tc.tile_pool(name=f"ps{layer}", bufs=4, space="PSUM"))

                hprev_f = hprev_b = None
                cprev = None
                n_chunks = T // C
                for ch in range(n_chunks):
                    t0 = ch * C
                    tb0 = T - C - t0  # bwd chunk start (ascending t)
                    pf = ppool.tile([128, 4, C * BL], BF16, tag="pf")
                    pb = ppool.tile([128, 4, C * BL], BF16, tag="pb")
                    nc.sync.dma_start(pf[:], projf[:, :, t0 * BL:(t0 + C) * BL])
                    nc.sync.dma_start(pb[:], projb[:, :, tb0 * BL:(tb0 + C) * BL])
                    hf_ch = hpool.tile([128, C * BL], BF16, tag="hf")
                    hb_ch = hpool.tile([128, C * BL], BF16, tag="hb")

                    for c in range(C):
                        step = t0 + c
                        cb = C - 1 - c  # bwd slot (reversed within chunk)
                        ps = psum.tile([128, 64], F32)
                        psv = ps[:].rearrange("p (d x) -> p d x", d=2)
                        # preload input projections into psum
                        nc.tensor.matmul(
                            ps[:, 0:32], idt[:],
                            pf[:, :, c * BL:(c + 1) * BL],
                            start=True, stop=False, skip_group_check=True)
                        nc.tensor.matmul(
                            ps[:, 32:64], idt[:],
                            pb[:, :, cb * BL:(cb + 1) * BL],
                            start=False, stop=(step == 0),
                            skip_group_check=True)
                        if step > 0:
                            # recurrent matmuls; g-gate (idx 3) first so ACT
                            # tanh can start early
                            mms = []
                            for g in (3, 0, 1, 2):
                                mms.append((whf, hprev_f, 0, g))
                                mms.append((whb, hprev_b, 1, g))
                            for k, (wsb, hap, d, g) in enumerate(mms):
                                nc.tensor.matmul(
                                    ps[:, d * 32 + g * 8: d * 32 + g * 8 + 8],
                                    wsb[:, g * 128:(g + 1) * 128], hap,
                                    start=False, stop=(k == len(mms) - 1),
                                    skip_group_check=True)
                        S = spool.tile([128, 2, 32], F32, tag="S")
                        # tanh(g-gate), sigmoid(i,f,o)
                        nc.scalar.activation(S[:, :, 24:32], psv[:, :, 24:32],
                                             AF.Tanh)
                        nc.scalar.activation(S[:, :, 0:24], psv[:, :, 0:24],
                                             AF.Sigmoid)
                        A = spool.tile([128, 2, 8], F32, tag="A")
                        nc.vector.tensor_tensor(
                            A[:], S[:, :, 0:8], S[:, :, 24:32], OP.mult)
                        cnew = cstp.tile([128, 2, 8], F32, tag="c")
                        if step > 0:
                            nc.vector.tensor_tensor(
                                cnew[:], S[:, :, 8:16], cprev[:], OP.mult)
                            nc.vector.tensor_tensor(cnew[:], cnew[:], A[:],
                                                    OP.add)
                        else:
                            nc.vector.tensor_copy(cnew[:], A[:])
                        TC = spool.tile([128, 2, 8], F32, tag="TC")
                        nc.scalar.activation(TC[:], cnew[:], AF.Tanh)
                        # h = sig(o) * tanh(c), written into chunk slots
                        hf_sl = hf_ch[:, c * BL:(c + 1) * BL]
                        hb_sl = hb_ch[:, cb * BL:(cb + 1) * BL]
                        nc.vector.tensor_tensor(
                            hf_sl, S[:, 0, 16:24], TC[:, 0, :], OP.mult)
                        nc.vector.tensor_tensor(
                            hb_sl, S[:, 1, 16:24], TC[:, 1, :], OP.mult)
                        hprev_f, hprev_b = hf_sl, hb_sl
                        cprev = cnew

                    nc.sync.dma_start(
                        out_ap[:, kb_f, t0 * BL:(t0 + C) * BL], hf_ch[:])
                    nc.sync.dma_start(
                        out_ap[:, kb_b, tb0 * BL:(tb0 + C) * BL], hb_ch[:])

        # ---- S0 ----
        scan(0, proj0["f"], proj0["b"], whh0["f"], whh0["b"], l0out, 0, 1)

        # ---- P2: layer-1 projections ----
        with ExitStack() as ctx:
            for d in "fb":
                matmul_tile_kernel(tc, w1[d][:], l0out[:], proj1[d][:])

        # ---- S1 ----
        scan(1, proj1["f"], proj1["b"], whh1["f"], whh1["b"], l1out, 0, 1)

        # ---- P3: head ----
        with ExitStack() as ctx:
            cpool = ctx.enter_context(tc.tile_pool(name="headc", bufs=1))
            lw_sb = cpool.tile([128, 2, 64], BF16)
            lb_sb = cpool.tile([64, 1], F32)
            w3_sb = cpool.tile([64, 1], BF16)
            nc.sync.dma_start(lw_sb[:], lwk[:])
            nc.sync.dma_start(lb_sb[:], lbv[:])
            nc.sync.dma_start(w3_sb[:], w3s[:])
            zpool = ctx.enter_context(tc.tile_pool(name="headz", bufs=3))
            upool = ctx.enter_context(tc.tile_pool(name="headu", bufs=3))
            dpool = ctx.enter_context(tc.tile_pool(name="headd", bufs=3))
            hps = ctx.enter_context(
                tc.tile_pool(name="headps", bufs=2, space="PSUM"))
            hps2 = ctx.enter_context(
                tc.tile_pool(name="headps2", bufs=2, space="PSUM"))
            for i in range(N // 512):
                zt = zpool.tile([128, 2, 512], BF16, tag="z")
                nc.sync.dma_start(zt[:], l1out[:, :, bass.ts(i, 512)])
                pu = hps.tile([64, 512], F32)
                nc.tensor.matmul(pu[:], lw_sb[:, 0, :], zt[:, 0, :],
                                 start=True, stop=False, skip_group_check=True)
                nc.tensor.matmul(pu[:], lw_sb[:, 1, :], zt[:, 1, :],
                                 start=False, stop=True, skip_group_check=True)
                ut = upool.tile([64, 512], BF16, tag="u")
                nc.scalar.activation(ut[:], pu[:], AF.Tanh, bias=lb_sb[:])
                pd = hps2.tile([1, 512], F32)
                nc.tensor.matmul(pd[:], w3_sb[:], ut[:])
                dt_ = dpool.tile([1, 512], F32, tag="d")
                nc.vector.tensor_copy(dt_[:], pd[:])
                # scatter [1, (t_sub, b)] -> dud[b, i*64 + t_sub]
                dst = dud[:, bass.ts(i, 64)].rearrange("b (o t) -> o t b", o=1)
                src = dt_[:].rearrange("o (t b) -> o t b", b=BL)
                nc.sync.dma_start(dst, src)

        # ---- P4: context solve (jacobi) + output ----
        with ExitStack() as ctx:
            cpool = ctx.enter_context(tc.tile_pool(name="ctxc", bufs=1))
            cf = cpool.tile([8, 8], F32)
            nc.sync.dma_start(cf[:], coef[:])
            d0 = cpool.tile([BL, T], F32)
            nc.sync.dma_start(d0[:], dud[:])
            # d0 += K0
            nc.vector.tensor_scalar(d0[:], d0[:], cf[:, 4:5], None, OP.add)
            jp = ctx.enter_context(tc.tile_pool(name="jac", bufs=2))
            sp_p = ctx.enter_context(tc.tile_pool(name="jsp", bufs=2))
            d_cur = d0
            g_, dl_, a_, b_ = (cf[:, 0:1], cf[:, 1:2], cf[:, 2:3], cf[:, 3:4])

            def stt(out, in0, scal, in1):
                nc.vector.scalar_tensor_tensor(out, in0, scal, in1,
                                               OP.mult, OP.add)

            def softplus(out_ap, in_ap):
                # Softplus has no ACT table on this build: ln(1 + exp(x)).
                # d stays small (|d| < ~3) so no overflow concerns.
                nc.scalar.activation(out_ap, in_ap, AF.Exp)
                nc.vector.tensor_scalar(out_ap, out_ap, 1.0, None, OP.add)
                nc.scalar.activation(out_ap, out_ap, AF.Ln)

            for it in range(N_JACOBI):
                sp = sp_p.tile([BL, T], F32, tag="sp")
                softplus(sp[:], d_cur[:])
                acc = jp.tile([BL, T], F32, tag="acc")
                nc.vector.tensor_copy(acc[:, 0:2], d0[:, 0:2])
                stt(acc[:, 1:T], d_cur[:, 0:T - 1], g_, d0[:, 1:T])
                stt(acc[:, 1:T], sp[:, 0:T - 1], dl_, acc[:, 1:T])
                stt(acc[:, 2:T], d_cur[:, 0:T - 2], a_, acc[:, 2:T])
                stt(acc[:, 2:T], sp[:, 0:T - 2], b_, acc[:, 2:T])
                d_cur = acc

            spf = sp_p.tile([BL, T], F32, tag="sp")
            softplus(spf[:], d_cur[:])
            lo = cpool.tile([BL, T * 2], F32)
            lov = lo[:].rearrange("p (t x) -> p t x", x=2)
            nc.vector.tensor_scalar(lov[:, :, 0], spf[:], -1.0, None, OP.mult)
            nc.vector.tensor_tensor(lov[:, :, 1], d_cur[:], spf[:],
                                    OP.subtract)
            out_view = outv.rearrange("(b t) x -> b t x", b=BL)
            nc.sync.dma_start(out_view, lov)

    nc.compile()
    _BUILD_CACHE[T] = nc
    return nc


# ---------------------------------------------------------------------------
# host-side prep + execution
# ---------------------------------------------------------------------------
def _prep_shared(inputs):
    sh = {}
    for l, (din_, kpad, wkey) in enumerate(((D_IN, 512, "w0"),
                                            (256, 384, "w1"))):
        for d, suf in (("f", ""), ("b", "r")):
            wih = _perm_gates(inputs[f"w_ih_l{l}{suf}"])       # [512, din]
            whh = _perm_gates(inputs[f"w_hh_l{l}{suf}"])       # [512, 128]
            bias = _perm_gates(
                np.asarray(inputs[f"b_ih_l{l}{suf}"])
                + np.asarray(inputs[f"b_hh_l{l}{suf}"]))       # [512]
            aug = np.zeros((kpad, 512), np.float32)
            aug[:din_] = np.asarray(wih, np.float32).T
            aug[din_] = bias
            sh[f"{wkey}{d}"] = _bf16(
                aug.reshape(kpad // 128, 128, 512).transpose(1, 0, 2))
            sh[f"whh{l}{d}"] = _bf16(np.asarray(whh, np.float32).T)

    g1, b1 = np.asarray(inputs["bn1_g"]), np.asarray(inputs["bn1_b"])
    m1, v1 = np.asarray(inputs["bn1_m"]), np.asarray(inputs["bn1_v"])
    s1 = g1 / np.sqrt(v1 + EPS)
    t1 = b1 - m1 * s1
    lin_w = np.asarray(inputs["lin_w"])
    LW = lin_w * s1[None, :]
    LB = np.asarray(inputs["lin_b"]) + lin_w @ t1
    g2, b2 = np.asarray(inputs["bn2_g"]), np.asarray(inputs["bn2_b"])
    m2, v2 = np.asarray(inputs["bn2_m"]), np.asarray(inputs["bn2_v"])
    s2 = g2 / np.sqrt(v2 + EPS)
    t2 = b2 - m2 * s2
    out_w, out_b = np.asarray(inputs["out_w"]), np.asarray(inputs["out_b"])
    W1, W2, W3 = out_w[:, 0:2], out_w[:, 2:4], out_w[:, 4:68]
    w3d = W3[1] - W3[0]
    K0 = (out_b[1] - out_b[0]) + t2 @ w3d
    w1d, w2d = W1[1] - W1[0], W2[1] - W2[0]
    alpha, beta = w1d[1], w1d[0] + w1d[1]
    gamma, delta = w2d[1], w2d[0] + w2d[1]

    sh["lwk"] = _bf16(LW.T.reshape(2, 128, 64).transpose(1, 0, 2))
    sh["lbv"] = np.asarray(LB, np.float32).reshape(64, 1)
    sh["w3s"] = _bf16((w3d * s2).reshape(64, 1))
    coefs = np.zeros((8, 8), np.float32)
    coefs[:, 0] = gamma
    coefs[:, 1] = -delta
    coefs[:, 2] = alpha
    coefs[:, 3] = -beta
    coefs[:, 4] = K0
    sh["coef"] = coefs
    sh["ident"] = _bf16(np.eye(128, dtype=np.float32))
    return sh


def _prep_core(x_core):
    # x_core: [BL, T, 400] -> aug kxn [128, 4, T*BL] bf16
    xt = np.zeros((512, T * BL), np.float32)
    xt[:D_IN] = np.asarray(x_core, np.float32).transpose(2, 1, 0).reshape(
        D_IN, T * BL)
    xt[D_IN] = 1.0
    return _bf16(xt.reshape(4, 128, T * BL).transpose(1, 0, 2))


def kernel(**inputs):
    nc = _build()
    sh = _prep_shared(inputs)
    x = np.asarray(inputs["x"], np.float32)
    in_maps = []
    for cidx in range(N_CORES):
        m = dict(sh)
        m["xk"] = _prep_core(x[cidx * BL:(cidx + 1) * BL])
        in_maps.append(m)
    res = run_bass_kernel_spmd(nc, in_maps, list(range(N_CORES)))
    outs = [np.asarray(res.results[i]["outv"], np.float32)
            for i in range(N_CORES)]
    return np.concatenate(outs, axis=0)


if __name__ == "__main__":
    import time
    t0 = time.time()
    print(f"building T={T}...")
    _build()
    print(f"built in {time.time() - t0:.1f}s")

